# revision 1
# baseline (speedup 1.0000x reference)
"""Multi-head attention (B=4, S=2048, D=1024, H=16, Dk=64) on 8 trn2 NeuronCores.

Sharding: core = (batch b, head-group g) with b in 0..3, g in 0..1.
Each core computes attention for its batch and its 8 heads, plus the partial
out-projection for its 512 columns of Wo.  Host sums the two partials per
batch and adds bo.

Per-core kernel (matmuls in float32r = TF32 fast mode, ~4e-4 rel err):
  phase A: PE-transpose q/k/v 128x128 tiles (f32r transpose mode); project to
           qhT/khT [c=512, s=2048] (c on partitions, pairs of heads per
           128-partition tile) and vh [s=2048, c] stored with a ones column
           per head ([128, 16, 8, 65] layout).  Biases are folded in as K=1
           outer-product matmuls opening each accumulation group.
  phase B (per 1024-wide query chunk, per head):
           scoresT[sk,sq] = khT_h^T @ qhT_h  (K=64 contraction)
           probsT = exp(scoresT/8 + maskbias[sk])   (mask folded into the
           ACT per-partition bias; masked keys underflow to exactly 0)
           attnT[c(+sums),sq] += vh_ext^T @ probsT  (ones column gives the
           softmax denominator in row 64 for free)
           normalize: approx-reciprocal of row 64, replicate across 64
           partitions with a K=1 outer-product matmul, multiply -> concatT
  phase C (interleaved per query chunk, after its 8 heads finish):
           out[sq,:] = concatT^T @ Wo  (accumulate 4 c-chunks in PSUM)
"""

import os
import sys

sys.path.insert(0, "/opt/trn_rl_repo")

import numpy as np

B, S, D, H, DK = 4, 2048, 1024, 16, 64
CPG = 512          # projection columns per core (8 heads x 64)
NCORES = 8

_cache = {}


def _build_nc():
    import concourse.bass as bass
    import concourse.tile as tile
    from concourse import bacc, mybir

    f32 = mybir.dt.float32
    R = mybir.dt.float32r
    Exp = mybir.ActivationFunctionType.Exp

    nc = bacc.Bacc("TRN2", target_bir_lowering=False, debug=False)

    q_d = nc.dram_tensor("q", [S, D], f32, kind="ExternalInput").ap()
    k_d = nc.dram_tensor("k", [S, D], f32, kind="ExternalInput").ap()
    v_d = nc.dram_tensor("v", [S, D], f32, kind="ExternalInput").ap()
    wq_d = nc.dram_tensor("wq", [D, CPG], R, kind="ExternalInput").ap()
    wk_d = nc.dram_tensor("wk", [D, CPG], R, kind="ExternalInput").ap()
    wv_d = nc.dram_tensor("wv", [D, CPG], R, kind="ExternalInput").ap()
    wo_d = nc.dram_tensor("wo", [CPG, D], R, kind="ExternalInput").ap()
    bq_d = nc.dram_tensor("bq", [CPG], R, kind="ExternalInput").ap()
    bk_d = nc.dram_tensor("bk", [CPG], R, kind="ExternalInput").ap()
    bv_d = nc.dram_tensor("bv", [CPG], R, kind="ExternalInput").ap()
    mb_d = nc.dram_tensor("maskbias", [128, 16], f32, kind="ExternalInput").ap()
    ones_d = nc.dram_tensor("ones", [128, 512], R, kind="ExternalInput").ap()
    ident_d = nc.dram_tensor("ident", [128, 128], f32, kind="ExternalInput").ap()
    out_d = nc.dram_tensor("out", [S, D], f32, kind="ExternalOutput").ap()

    NSQ = S // 512       # 4 query/key 512-blocks
    NDCH = D // 128      # 8 contraction chunks for projections
    NSK = S // 128       # 16 key chunks
    NPAIR = 4            # head pairs per core

    with tile.TileContext(nc) as tc:
        import contextlib

        with contextlib.ExitStack() as ctx:
            # ---------- persistent tensors + constants ----------
            persist = ctx.enter_context(tc.tile_pool(name="persist", bufs=1))
            consts = ctx.enter_context(tc.tile_pool(name="consts", bufs=1))

            qhT_sb = persist.tile([128, NPAIR, S], R)   # [c%128, pair, sq]
            khT_sb = persist.tile([128, NPAIR, S], R)
            vh_sb = persist.tile([128, NSK, 8, DK + 1], R)  # ones col at 64

            ones_sb = consts.tile([1, 512], R)
            nc.sync.dma_start(out=ones_sb, in_=ones_d[0:1, :])
            nc.sync.dma_start(
                out=vh_sb[:, :, :, DK],
                in_=ones_d[:, 0:128].rearrange("p (a b) -> p a b", a=16),
            )
            mb_sb = consts.tile([128, 16], f32)
            nc.sync.dma_start(out=mb_sb, in_=mb_d)
            wo_sb = consts.tile([128, NPAIR, D], R)
            for j in range(NPAIR):
                nc.sync.dma_start(
                    out=wo_sb[:, j, :], in_=wo_d[j * 128 : j * 128 + 128, :]
                )

            # ---------- phase A: transposes + projections ----------
            with contextlib.ExitStack() as actx:
                aconsts = actx.enter_context(tc.tile_pool(name="aconsts", bufs=1))
                wpool = actx.enter_context(tc.tile_pool(name="wpool", bufs=2))
                natpool = actx.enter_context(tc.tile_pool(name="natpool", bufs=6))
                xtpool = actx.enter_context(tc.tile_pool(name="xtpool", bufs=3))
                tppool = actx.enter_context(
                    tc.tile_pool(name="tppool", bufs=2, space="PSUM")
                )
                prpool = actx.enter_context(
                    tc.tile_pool(name="prpool", bufs=4, space="PSUM")
                )

                ident = aconsts.tile([128, 128], f32)
                nc.sync.dma_start(out=ident, in_=ident_d)
                bq_sb = aconsts.tile([1, CPG], R)
                nc.sync.dma_start(out=bq_sb, in_=bq_d[None, :])
                bk_sb = aconsts.tile([1, CPG], R)
                nc.sync.dma_start(out=bk_sb, in_=bk_d[None, :])
                bv_sb = aconsts.tile([1, CPG], R)
                nc.sync.dma_start(out=bv_sb, in_=bv_d[None, :])

                for tname, x_d, w_d, b_sb in (
                    ("q", q_d, wq_d, bq_sb),
                    ("k", k_d, wk_d, bk_sb),
                    ("v", v_d, wv_d, bv_sb),
                ):
                    w_sb = wpool.tile([128, NDCH, CPG], R, tag="w")
                    first_nats = []
                    if tname == "q":
                        for i in range(4):
                            x_nat = natpool.tile([128, D], f32, tag="nat")
                            nc.sync.dma_start(out=x_nat, in_=x_d[i * 128 : i * 128 + 128, :])
                            first_nats.append(x_nat)
                    for j in range(NDCH):
                        nc.sync.dma_start(
                            out=w_sb[:, j, :], in_=w_d[j * 128 : j * 128 + 128, :]
                        )
                    for sq in range(NSQ):
                        if sq == 0 and first_nats:
                            nats = first_nats
                        else:
                            nats = []
                            for i in range(4):
                                x_nat = natpool.tile([128, D], f32, tag="nat")
                                r0 = sq * 512 + i * 128
                                nc.sync.dma_start(out=x_nat, in_=x_d[r0 : r0 + 128, :])
                                nats.append(x_nat)

                        # open accumulation groups: bias outer-product first
                        prs = []
                        for cch in range(4):
                            pr = prpool.tile([128, 512], f32, tag="pr")
                            prs.append(pr)
                            if tname == "v":
                                nc.tensor.matmul(
                                    pr,
                                    lhsT=ones_sb[0:1, 0:128],
                                    rhs=b_sb[0:1, :],
                                    start=True,
                                    stop=False,
                                )
                            else:
                                nc.tensor.matmul(
                                    pr,
                                    lhsT=b_sb[0:1, cch * 128 : cch * 128 + 128],
                                    rhs=ones_sb[0:1, 0:512],
                                    start=True,
                                    stop=False,
                                )

                        for j in range(NDCH):
                            tp = tppool.tile([128, 512], f32, tag="tp")
                            for i in range(4):
                                nc.tensor.transpose(
                                    out=tp[:, i * 128 : i * 128 + 128],
                                    in_=nats[i][:, j * 128 : j * 128 + 128],
                                    identity=ident,
                                )
                            xt = xtpool.tile([128, 512], R, tag="xt")
                            nc.scalar.copy(out=xt, in_=tp)
                            for cch in range(4):
                                if tname == "v":
                                    # vh[sk,c]: lhsT = xT chunk, rhs = W chunk
                                    nc.tensor.matmul(
                                        prs[cch],
                                        lhsT=xt[:, cch * 128 : cch * 128 + 128],
                                        rhs=w_sb[:, j, :],
                                        start=False,
                                        stop=(j == NDCH - 1),
                                    )
                                else:
                                    # qhT[c,sq]: lhsT = W chunk, rhs = xT
                                    nc.tensor.matmul(
                                        prs[cch],
                                        lhsT=w_sb[:, j, cch * 128 : cch * 128 + 128],
                                        rhs=xt,
                                        start=False,
                                        stop=(j == NDCH - 1),
                                    )

                        for cch in range(4):
                            if tname == "v":
                                skc = sq * 4 + cch
                                nc.vector.tensor_copy(
                                    out=vh_sb[:, skc, :, 0:DK],
                                    in_=prs[cch].rearrange("p (h d) -> p h d", h=8),
                                )
                            else:
                                dst = qhT_sb if tname == "q" else khT_sb
                                nc.vector.tensor_copy(
                                    out=dst[:, cch, sq * 512 : sq * 512 + 512],
                                    in_=prs[cch],
                                )

            # ---------- phase B: attention ----------
            concpool = ctx.enter_context(tc.tile_pool(name="concpool", bufs=1))
            concatT_sb = concpool.tile([128, NPAIR, S], R)
            with contextlib.ExitStack() as bctx:
                probpool = bctx.enter_context(tc.tile_pool(name="probpool", bufs=3))
                npool = bctx.enter_context(tc.tile_pool(name="npool", bufs=2))
                rppool = bctx.enter_context(tc.tile_pool(name="rppool", bufs=1))
                scpool = bctx.enter_context(
                    tc.tile_pool(name="scpool", bufs=2, space="PSUM")
                )
                atpool = bctx.enter_context(
                    tc.tile_pool(name="atpool", bufs=3, space="PSUM")
                )
                reppool = bctx.enter_context(
                    tc.tile_pool(name="reppool", bufs=1, space="PSUM")
                )

                for sq2 in range(S // 1024):
                    for pair in range(NPAIR):
                        for hh in range(2):
                            h = pair * 2 + hh
                            base = hh * 64
                            at_ps0 = atpool.tile([128, 512], f32, tag="at")
                            at_ps1 = atpool.tile([128, 512], f32, tag="at")
                            at_halves = (at_ps0, at_ps1)
                            for sk in range(NSK):
                                sc_ps = scpool.tile([128, 1024], f32, tag="sc")
                                for half in range(2):
                                    qoff = sq2 * 1024 + half * 512
                                    nc.tensor.matmul(
                                        sc_ps[:, half * 512 : half * 512 + 512],
                                        lhsT=khT_sb[
                                            base : base + 64,
                                            pair,
                                            sk * 128 : sk * 128 + 128,
                                        ],
                                        rhs=qhT_sb[
                                            base : base + 64, pair, qoff : qoff + 512
                                        ],
                                        start=True,
                                        stop=True,
                                    )
                                probs = probpool.tile([128, 1024], R, tag="probs")
                                nc.scalar.activation(
                                    out=probs,
                                    in_=sc_ps,
                                    func=Exp,
                                    bias=mb_sb[:, sk : sk + 1],
                                    scale=0.125,
                                )
                                for half in range(2):
                                    nc.tensor.matmul(
                                        at_halves[half][0:65, :],
                                        lhsT=vh_sb[:, sk, h, :],
                                        rhs=probs[:, half * 512 : half * 512 + 512],
                                        start=(sk == 0),
                                        stop=(sk == NSK - 1),
                                    )
                            attn_sb = npool.tile([128, 1024], f32, tag="attn")
                            for half in range(2):
                                nc.vector.tensor_copy(
                                    out=attn_sb[
                                        0:65, half * 512 : half * 512 + 512
                                    ],
                                    in_=at_halves[half][0:65, :],
                                )
                            recip32 = rppool.tile([1, 1024], f32, tag="recip32")
                            nc.vector.reciprocal(recip32, attn_sb[64:65, :])
                            recip = rppool.tile([1, 1024], R, tag="recip")
                            nc.vector.tensor_copy(out=recip, in_=recip32)
                            for half in range(2):
                                rep_ps = reppool.tile([64, 512], f32, tag="rep")
                                nc.tensor.matmul(
                                    rep_ps,
                                    lhsT=ones_sb[0:1, 0:64],
                                    rhs=recip[0:1, half * 512 : half * 512 + 512],
                                    start=True,
                                    stop=True,
                                )
                                nc.vector.tensor_mul(
                                    concatT_sb[
                                        base : base + 64,
                                        pair,
                                        sq2 * 1024 + half * 512 : sq2 * 1024
                                        + half * 512
                                        + 512,
                                    ],
                                    attn_sb[0:64, half * 512 : half * 512 + 512],
                                    rep_ps,
                                )

            # ---------- phase C: out projection ----------
            with contextlib.ExitStack() as cctx:
                outpool = cctx.enter_context(
                    tc.tile_pool(name="outpool", bufs=3)
                )
                opspool = cctx.enter_context(
                    tc.tile_pool(name="opspool", bufs=4, space="PSUM")
                )
                for sqc in range(S // 128):
                    for do in range(2):
                        o_ps = opspool.tile([128, 512], f32, tag="ops")
                        for j in range(NPAIR):
                            nc.tensor.matmul(
                                o_ps,
                                lhsT=concatT_sb[
                                    :, j, sqc * 128 : sqc * 128 + 128
                                ],
                                rhs=wo_sb[:, j, do * 512 : do * 512 + 512],
                                start=(j == 0),
                                stop=(j == NPAIR - 1),
                            )
                        o_sb = outpool.tile([128, 512], f32, tag="osb")
                        nc.vector.tensor_copy(out=o_sb, in_=o_ps)
                        nc.sync.dma_start(
                            out=out_d[
                                sqc * 128 : sqc * 128 + 128,
                                do * 512 : do * 512 + 512,
                            ],
                            in_=o_sb,
                        )

    nc.compile()
    return nc


def get_nc():
    if "nc" not in _cache:
        _cache["nc"] = _build_nc()
    return _cache["nc"]


def make_in_maps(q, k, v, mask, Wq, bq, Wk, bk, Wv, bv, Wo, bo):
    f32 = np.float32
    c = np.ascontiguousarray
    in_maps = []
    for core in range(NCORES):
        b, g = core // 2, core % 2
        cols = slice(g * CPG, (g + 1) * CPG)
        mb = (-1e9 * (1.0 - np.asarray(mask[b, 0], f32))).reshape(16, 128).T
        in_maps.append(
            {
                "q": c(np.asarray(q[b], f32)),
                "k": c(np.asarray(k[b], f32)),
                "v": c(np.asarray(v[b], f32)),
                "wq": c(np.asarray(Wq[:, cols], f32)),
                "wk": c(np.asarray(Wk[:, cols], f32)),
                "wv": c(np.asarray(Wv[:, cols], f32)),
                "wo": c(np.asarray(Wo[cols, :], f32)),
                "bq": c(np.asarray(bq[cols], f32)),
                "bk": c(np.asarray(bk[cols], f32)),
                "bv": c(np.asarray(bv[cols], f32)),
                "maskbias": c(mb),
                "ones": np.ones((128, 512), f32),
                "ident": np.eye(128, dtype=f32),
            }
        )
    return in_maps


def gather(results, bo):
    out = np.zeros((B, S, D), np.float32)
    for core in range(NCORES):
        b = core // 2
        out[b] += results[core]["out"]
    out += np.asarray(bo, np.float32)[None, None, :]
    return out


def run_on_hw(in_maps, trace=False, trace_cores=None):
    from concourse.bass_utils import run_bass_kernel_spmd

    nc = get_nc()
    return run_bass_kernel_spmd(
        nc,
        in_maps,
        list(range(NCORES)),
        trace=trace,
        trace_cores=trace_cores,
    )


def kernel(q, k, v, mask, Wq, bq, Wk, bk, Wv, bv, Wo, bo):
    in_maps = make_in_maps(q, k, v, mask, Wq, bq, Wk, bk, Wv, bv, Wo, bo)
    res = run_on_hw(in_maps)
    return gather(res.results, bo)



# revision 6
# speedup vs baseline: 2.3478x; 2.3478x over previous
"""Multi-head attention (B=4, S=2048, D=1024, H=16, Dk=64) on 8 trn2 NeuronCores.

Sharding: core = (batch b, head-group g), b in 0..3, g in 0..1.  Each core
computes attention for its batch and its 8 heads plus the partial out
projection for its 512 rows of Wo; host sums the two partials per batch and
adds bo.

Key optimizations over the naive version:
  - Host-side key compaction: mask keys (~50% zeros) are gathered out of k/v
    before upload, so the device only scores/exps/attends over valid keys
    (padded to a multiple of 128; pad lanes get a -1e9 exp bias -> probs 0).
  - Host-side transposes + bf16 casts: q/k/v arrive as [D, S] bf16, so phase A
    needs no PE transposes, weight loads use FWL, and DMA bytes halve.
  - Scores for a head PAIR run as two concurrent row-tiled K=64 matmuls
    (partition bases 0/64) into one PSUM tile, so a single [128,1024] ACT exp
    covers both heads (ACT is the bottleneck engine; fewer+wider ACTIVATEs).
  - The ones-column in vh yields softmax denominators for free (row 64 of the
    attn PSUM); reciprocal via the fast approx DVE op; recip broadcast across
    64 partitions with a K=1 matmul; normalize straight out of PSUM.
  - Software-pipelined emission (engine queues are FIFO): scores(sk+1) is
    emitted before attn(sk); normalization of the previous pair and the out
    projection of the previous query block are emitted as PE fillers early in
    the next pair's key loop.
"""

import sys

sys.path.insert(0, "/opt/trn_rl_repo")

import numpy as np

B, S, D, H, DK = 4, 2048, 1024, 16, 64
CPG = 512          # projection columns per core (8 heads x 64)
NPAIR = 4          # head pairs per core
NDCH = D // 128    # contraction chunks for projections
NCORES = 8

_cache = {}


def _build_nc(nskv, zero_bias):
    import contextlib

    import concourse.bass as bass
    import concourse.tile as tile
    from concourse import bacc, mybir

    f32 = mybir.dt.float32
    bf16 = mybir.dt.bfloat16
    Exp = mybir.ActivationFunctionType.Exp

    skv = nskv * 128

    nc = bacc.Bacc("TRN2", target_bir_lowering=False, debug=False)

    qt_d = nc.dram_tensor("qt", [D, S], bf16, kind="ExternalInput").ap()
    kt_d = nc.dram_tensor("kt", [D, skv], bf16, kind="ExternalInput").ap()
    vt_d = nc.dram_tensor("vt", [D, skv], bf16, kind="ExternalInput").ap()
    wq_d = nc.dram_tensor("wq", [D, CPG], bf16, kind="ExternalInput").ap()
    wk_d = nc.dram_tensor("wk", [D, CPG], bf16, kind="ExternalInput").ap()
    wv_d = nc.dram_tensor("wv", [D, CPG], bf16, kind="ExternalInput").ap()
    wo_d = nc.dram_tensor("wo", [CPG, D], bf16, kind="ExternalInput").ap()
    mb_d = nc.dram_tensor("maskbias", [128, nskv], f32, kind="ExternalInput").ap()
    ones_d = nc.dram_tensor("ones", [128, 512], bf16, kind="ExternalInput").ap()
    if not zero_bias:
        bq_d = nc.dram_tensor("bq", [128, NPAIR], f32, kind="ExternalInput").ap()
        bk_d = nc.dram_tensor("bk", [128, NPAIR], f32, kind="ExternalInput").ap()
        bv_d = nc.dram_tensor("bv", [1, CPG], bf16, kind="ExternalInput").ap()
    out_d = nc.dram_tensor("out", [S, D], f32, kind="ExternalOutput").ap()

    # skv split into 512-wide column chunks for the khT projection copies
    kq_chunks = []
    o = 0
    while o < skv:
        w = min(512, skv - o)
        kq_chunks.append((o, w))
        o += w

    with tile.TileContext(nc) as tc:
        with contextlib.ExitStack() as ctx:
            # ---------- persistent tensors + constants ----------
            persist = ctx.enter_context(tc.tile_pool(name="persist", bufs=1))
            consts = ctx.enter_context(tc.tile_pool(name="consts", bufs=1))

            qhT_sb = persist.tile([128, NPAIR, S], bf16)      # [c%128, pair, sq]
            khT_sb = persist.tile([128, NPAIR, skv], bf16)    # [c%128, pair, sk]
            vh_sb = persist.tile([128, nskv, 8, DK + 1], bf16)  # ones col at DK
            concatT_sb = persist.tile([128, NPAIR, S], bf16)

            ones_sb = consts.tile([1, 512], bf16)
            nc.sync.dma_start(out=ones_sb, in_=ones_d[0:1, :])
            nc.sync.dma_start(
                out=vh_sb[:, :, :, DK],
                in_=ones_d[:, 0 : nskv * 8].rearrange("p (a b) -> p a b", a=nskv),
            )
            mb_sb = consts.tile([128, nskv], f32)
            nc.sync.dma_start(out=mb_sb, in_=mb_d)
            wo_sb = consts.tile([128, NPAIR, D], bf16)
            for j in range(NPAIR):
                nc.sync.dma_start(
                    out=wo_sb[:, j, :], in_=wo_d[j * 128 : j * 128 + 128, :]
                )
            if not zero_bias:
                bq_sb = consts.tile([128, NPAIR], f32)
                nc.sync.dma_start(out=bq_sb, in_=bq_d)
                bk_sb = consts.tile([128, NPAIR], f32)
                nc.sync.dma_start(out=bk_sb, in_=bk_d)
                bv_sb = consts.tile([1, CPG], bf16)
                nc.sync.dma_start(out=bv_sb, in_=bv_d)

            # ---------- phase A: projections (no transposes needed) ----------
            with contextlib.ExitStack() as actx:
                xpool = actx.enter_context(tc.tile_pool(name="xpool", bufs=1))
                wpool = actx.enter_context(tc.tile_pool(name="wpool", bufs=1))
                prpool = actx.enter_context(
                    tc.tile_pool(name="prpool", bufs=8, space="PSUM")
                )

                # k projection: khT[c, sk] = Wk^T @ kT
                kt_sb = xpool.tile([128, NDCH, skv], bf16, tag="kt")
                wk_sb = wpool.tile([128, NDCH, CPG], bf16, tag="wk")
                for j in range(NDCH):
                    nc.sync.dma_start(out=kt_sb[:, j, :], in_=kt_d[j * 128 : j * 128 + 128, :])
                    nc.sync.dma_start(out=wk_sb[:, j, :], in_=wk_d[j * 128 : j * 128 + 128, :])
                for o, w in kq_chunks:
                    prs = [prpool.tile([128, 512], f32, tag="pr", name=f"pr{i}") for i in range(4)]
                    for j in range(NDCH):
                        for cch in range(4):
                            nc.tensor.matmul(
                                prs[cch][:, 0:w],
                                lhsT=wk_sb[:, j, cch * 128 : cch * 128 + 128],
                                rhs=kt_sb[:, j, o : o + w],
                                start=(j == 0),
                                stop=(j == NDCH - 1),
                            )
                    for cch in range(4):
                        if zero_bias:
                            nc.vector.tensor_copy(
                                out=khT_sb[:, cch, o : o + w], in_=prs[cch][:, 0:w]
                            )
                        else:
                            nc.vector.tensor_scalar_add(
                                khT_sb[:, cch, o : o + w],
                                prs[cch][:, 0:w],
                                bk_sb[:, cch : cch + 1],
                            )

                # v projection: vh[sk, c] = vT^T @ Wv  (per 128-key chunk)
                vt_sb = xpool.tile([128, NDCH, skv], bf16, tag="vt")
                wv_sb = wpool.tile([128, NDCH, CPG], bf16, tag="wv")
                for j in range(NDCH):
                    nc.sync.dma_start(out=vt_sb[:, j, :], in_=vt_d[j * 128 : j * 128 + 128, :])
                    nc.sync.dma_start(out=wv_sb[:, j, :], in_=wv_d[j * 128 : j * 128 + 128, :])
                for skc in range(nskv):
                    pr = prpool.tile([128, 512], f32, tag="pr")
                    for j in range(NDCH):
                        if not zero_bias and j == 0:
                            nc.tensor.matmul(
                                pr,
                                lhsT=ones_sb[0:1, 0:128],
                                rhs=bv_sb[0:1, :],
                                start=True,
                                stop=False,
                            )
                        nc.tensor.matmul(
                            pr,
                            lhsT=vt_sb[:, j, skc * 128 : skc * 128 + 128],
                            rhs=wv_sb[:, j, :],
                            start=(zero_bias and j == 0),
                            stop=(j == NDCH - 1),
                        )
                    nc.vector.tensor_copy(
                        out=vh_sb[:, skc, :, 0:DK],
                        in_=pr.rearrange("p (h d) -> p h d", h=8),
                    )

                # q projection: qhT[c, sq] = Wq^T @ qT
                qt_sb = xpool.tile([128, NDCH, S], bf16, tag="qt")
                wq_sb = wpool.tile([128, NDCH, CPG], bf16, tag="wq")
                for j in range(NDCH):
                    nc.sync.dma_start(out=qt_sb[:, j, :], in_=qt_d[j * 128 : j * 128 + 128, :])
                    nc.sync.dma_start(out=wq_sb[:, j, :], in_=wq_d[j * 128 : j * 128 + 128, :])
                for sqq in range(4):
                    o = sqq * 512
                    prs = [prpool.tile([128, 512], f32, tag="pr", name=f"pr{i}") for i in range(4)]
                    for j in range(NDCH):
                        for cch in range(4):
                            nc.tensor.matmul(
                                prs[cch],
                                lhsT=wq_sb[:, j, cch * 128 : cch * 128 + 128],
                                rhs=qt_sb[:, j, o : o + 512],
                                start=(j == 0),
                                stop=(j == NDCH - 1),
                            )
                    for cch in range(4):
                        if zero_bias:
                            nc.vector.tensor_copy(
                                out=qhT_sb[:, cch, o : o + 512], in_=prs[cch]
                            )
                        else:
                            nc.vector.tensor_scalar_add(
                                qhT_sb[:, cch, o : o + 512],
                                prs[cch],
                                bq_sb[:, cch : cch + 1],
                            )

            # ---------- phase B + C: attention, fused with out projection ----
            with contextlib.ExitStack() as bctx:
                probpool = bctx.enter_context(tc.tile_pool(name="probpool", bufs=3))
                rc32pool = bctx.enter_context(tc.tile_pool(name="rc32pool", bufs=2))
                rc16pool = bctx.enter_context(tc.tile_pool(name="rc16pool", bufs=2))
                outpool = bctx.enter_context(tc.tile_pool(name="outpool", bufs=3))
                scpool = bctx.enter_context(
                    tc.tile_pool(name="scpool", bufs=2, space="PSUM")
                )
                atpool = bctx.enter_context(
                    tc.tile_pool(name="atpool", bufs=2, space="PSUM")
                )
                auxpool = bctx.enter_context(
                    tc.tile_pool(name="auxpool", bufs=2, space="PSUM")
                )

                def emit_norm(sqb, pair, atA, atB):
                    """Normalize both heads of a finished pair into concatT."""
                    q0 = sqb * 512
                    for hh, at in ((0, atA), (1, atB)):
                        base = hh * 64
                        r32 = rc32pool.tile([1, 512], f32, tag="r32")
                        nc.vector.reciprocal(r32, at[64:65, :])
                        rep = rc16pool.tile([64, 512], f32, tag="rep")
                        nc.gpsimd.partition_broadcast(rep, r32[0:1, :])
                        nc.vector.tensor_mul(
                            concatT_sb[base : base + 64, pair, q0 : q0 + 512],
                            at[0:64, :],
                            rep,
                        )

                def emit_outproj(sqb, sqc):
                    """One 128-query chunk of the out projection + store."""
                    q0 = sqb * 512 + sqc * 128
                    for do in range(2):
                        ops = auxpool.tile([128, 512], f32, tag="aux")
                        for j in range(NPAIR):
                            nc.tensor.matmul(
                                ops,
                                lhsT=concatT_sb[:, j, q0 : q0 + 128],
                                rhs=wo_sb[:, j, do * 512 : do * 512 + 512],
                                start=(j == 0),
                                stop=(j == NPAIR - 1),
                            )
                        osb = outpool.tile([128, 512], f32, tag="osb")
                        nc.vector.tensor_copy(out=osb, in_=ops)
                        nc.sync.dma_start(
                            out=out_d[q0 : q0 + 128, do * 512 : do * 512 + 512],
                            in_=osb,
                        )

                # fillers are (closure, ) lists emitted inside the NEXT pair's
                # sk loop, after a couple of score/exp stages are in flight
                pending = []

                for sqb in range(4):
                    for pair in range(NPAIR):
                        hA, hB = 2 * pair, 2 * pair + 1
                        q0 = sqb * 512
                        atA = atpool.tile([128, 512], f32, tag="at")
                        atB = atpool.tile([128, 512], f32, tag="at")

                        sc_tiles = [None] * nskv
                        probs_tiles = [None] * nskv

                        def emit_scores_exp(sk):
                            sc = scpool.tile([128, 1024], f32, tag="sc")
                            sc_tiles[sk] = sc
                            nc.tensor.matmul(
                                sc[:, 0:512],
                                lhsT=khT_sb[0:64, pair, sk * 128 : sk * 128 + 128],
                                rhs=qhT_sb[0:64, pair, q0 : q0 + 512],
                                start=True,
                                stop=True,
                            )
                            nc.tensor.matmul(
                                sc[:, 512:1024],
                                lhsT=khT_sb[64:128, pair, sk * 128 : sk * 128 + 128],
                                rhs=qhT_sb[64:128, pair, q0 : q0 + 512],
                                start=True,
                                stop=True,
                            )
                            probs = probpool.tile([128, 1024], bf16, tag="probs")
                            probs_tiles[sk] = probs
                            nc.scalar.activation(
                                out=probs,
                                in_=sc,
                                func=Exp,
                                bias=mb_sb[:, sk : sk + 1],
                                scale=0.125,
                            )

                        def emit_attn(sk):
                            probs = probs_tiles[sk]
                            nc.tensor.matmul(
                                atA[0:65, :],
                                lhsT=vh_sb[:, sk, hA, :],
                                rhs=probs[:, 0:512],
                                start=(sk == 0),
                                stop=(sk == nskv - 1),
                            )
                            nc.tensor.matmul(
                                atB[0:65, :],
                                lhsT=vh_sb[:, sk, hB, :],
                                rhs=probs[:, 512:1024],
                                start=(sk == 0),
                                stop=(sk == nskv - 1),
                            )

                        # software pipeline: scores(sk) runs one stage ahead
                        # of attn(sk); pending fillers (prev pair's norm, prev
                        # sqb's out-proj chunk) drop in after stage 2's scores
                        emit_scores_exp(0)
                        for sk in range(1, nskv):
                            emit_scores_exp(sk)
                            if sk == 2:
                                for f in pending:
                                    f()
                                pending = []
                            emit_attn(sk - 1)
                        emit_attn(nskv - 1)

                        pending.append(
                            lambda sqb=sqb, pair=pair, atA=atA, atB=atB: emit_norm(
                                sqb, pair, atA, atB
                            )
                        )
                        if sqb > 0:
                            pending.append(
                                lambda sqb=sqb, pair=pair: emit_outproj(
                                    sqb - 1, pair
                                )
                            )

                # drain: last pair's norm + all of sqb 3's out projection
                for f in pending:
                    f()
                for sqc in range(4):
                    emit_outproj(3, sqc)

    nc.compile()
    return nc


def get_nc(nskv=9, zero_bias=True):
    key = (nskv, zero_bias)
    if key not in _cache:
        _cache[key] = _build_nc(nskv, zero_bias)
    return _cache[key]


def make_in_maps(q, k, v, mask, Wq, bq, Wk, bk, Wv, bv, Wo, bo):
    import ml_dtypes

    f32 = np.float32
    bf16 = ml_dtypes.bfloat16
    c = np.ascontiguousarray

    mask = np.asarray(mask)
    idxs = [np.nonzero(mask[b, 0] != 0)[0] for b in range(B)]
    kvs = [len(ix) for ix in idxs]
    nskv = max(1, (max(kvs) + 127) // 128)
    skv = nskv * 128

    zero_bias = (
        not np.any(np.asarray(bq))
        and not np.any(np.asarray(bk))
        and not np.any(np.asarray(bv))
    )

    Wq, Wk, Wv, Wo = (np.asarray(a, f32) for a in (Wq, Wk, Wv, Wo))

    in_maps = []
    for core in range(NCORES):
        b, g = core // 2, core % 2
        cols = slice(g * CPG, (g + 1) * CPG)
        ix = idxs[b]
        kv = kvs[b]

        kc = np.zeros((skv, D), f32)
        vc = np.zeros((skv, D), f32)
        kc[:kv] = np.asarray(k[b], f32)[ix]
        vc[:kv] = np.asarray(v[b], f32)[ix]

        mbflat = np.where(np.arange(skv) < kv, 0.0, -1e9).astype(f32)

        m = {
            "qt": c(np.asarray(q[b], f32).T.astype(bf16)),
            "kt": c(kc.T.astype(bf16)),
            "vt": c(vc.T.astype(bf16)),
            "wq": c(Wq[:, cols].astype(bf16)),
            "wk": c(Wk[:, cols].astype(bf16)),
            "wv": c(Wv[:, cols].astype(bf16)),
            "wo": c(Wo[cols, :].astype(bf16)),
            "maskbias": c(mbflat.reshape(nskv, 128).T),
            "ones": np.ones((128, 512), bf16),
        }
        if not zero_bias:
            m["bq"] = c(np.asarray(bq, f32)[cols].reshape(NPAIR, 128).T)
            m["bk"] = c(np.asarray(bk, f32)[cols].reshape(NPAIR, 128).T)
            m["bv"] = c(np.asarray(bv, f32)[cols].reshape(1, CPG).astype(bf16))
        in_maps.append(m)
    return in_maps, nskv, zero_bias


def gather(results, bo):
    out = np.zeros((B, S, D), np.float32)
    for core in range(NCORES):
        b = core // 2
        out[b] += results[core]["out"]
    out += np.asarray(bo, np.float32)[None, None, :]
    return out


def run_on_hw(in_maps, nskv, zero_bias, trace=False, trace_cores=None):
    from concourse.bass_utils import run_bass_kernel_spmd

    nc = get_nc(nskv, zero_bias)
    return run_bass_kernel_spmd(
        nc,
        in_maps,
        list(range(NCORES)),
        trace=trace,
        trace_cores=trace_cores,
    )


def kernel(q, k, v, mask, Wq, bq, Wk, bk, Wv, bv, Wo, bo):
    in_maps, nskv, zero_bias = make_in_maps(
        q, k, v, mask, Wq, bq, Wk, bk, Wv, bv, Wo, bo
    )
    res = run_on_hw(in_maps, nskv, zero_bias)
    return gather(res.results, bo)
